# revision 12
# baseline (speedup 1.0000x reference)
# Bass/Tile TRN2 kernel for nn_Conv1D_style: out = ((x * (cluster@style_L)) @ weight) * (cluster@style_R)
#
# Sharding: data-parallel over the batch dim. Each of the 8 cores gets a
# 1024-row slice of x/cluster and a full (replicated) weight/style_L/style_R.
#
# Per-core plan (M=1024 batch, K=4096 din, N=4096 dout), all matmuls bf16
# with fp32 PSUM accumulation:
#   aT[k] = xT[k] * (style_L[:, kslice].T @ clusterT)  -> bf16, SBUF-resident.
#   y[m,n] = sum_k aT[k][:, mslice].T @ W[k, nslice]   (32 accumulating MMs)
#   out[m,n] = y[m,n] * (clusterT[:, mslice].T @ style_R[:, nslice])
#
# Schedule: the aT production is software-pipelined with the n=0
# accumulation for m0-5 (6 PSUM accumulators, k-outer): per k the PE runs
# [tmpLT pair k][6 fused MMs for k-1], so the vector at-mul for k (1.29us)
# hides under the fused MM group (1.30us) with pl bufs=1 acting as the
# pipeline interlock. All loads stream on the Sync HWDGE queue in exact
# consumption order with fine granules (xT in 2-k-tile granules, W(n=0) in
# 4-k-tile granule tiles, style_L in 8-k-tile granules) so the first PE op
# is gated on ~0.5 MiB, not 13 MiB; w1/w2 prefetch queues strictly after
# the prologue bytes. Output stores (and nothing else) issue on the
# Scalar/Activation queue so stores never delay loads.
#
# The K=64 style matmuls (tmpLT/tmpR) are row-packed two at a time via
# tile_position into the upper/lower 64 PE rows: the host ships
# cluster/style operands duplicated across partitions 0-63 and 64-127, and
# each packed pair costs one ~320ns slot instead of two.

import numpy as np
import ml_dtypes

B, DIN, DOUT, NCL = 8192, 4096, 4096, 64
NCORES = 8
MB = B // NCORES          # batch rows per core
P = 128
NT = 512                  # n tile (dout cols per matmul)
KT = DIN // P             # 32 k tiles
MT = MB // P              # 8 m tiles
NTS = DOUT // NT          # 8 n tiles
FUSED = 6                 # m tiles of n=0 accumulated during the aT prologue
XG = 2                    # k tiles per xT DMA granule (16 granules)
W0G = 4                   # k tiles per W(n=0) granule tile (8 granules)
SLG = 8                   # k tiles per style_L granule (4 granules)
FP8K = 6                  # k tiles 26..31 run as fp8e4 DoubleRow pairs
KC = KT - FP8K            # bf16 k tiles (0..KC-1)
NPR = FP8K // 2           # DoubleRow pair count (3)
# processing order: fp8 pairs (PE-light vs the at-mul window) interleaved
# between PE-heavy bf16 steps so the PE work queue never drains
PROC_KS = (list(range(0, 20))
           + [26, 27, 20, 21, 28, 29, 22, 23, 30, 31, 24, 25])

_CACHE = {}
LAST = {}                 # exposes the most recent BassKernelResults for test harnesses


def _build_program():
    import concourse.bacc as bacc
    import concourse.mybir as mybir
    import concourse.tile as tile

    bf16 = mybir.dt.bfloat16
    f32 = mybir.dt.float32

    nc = bacc.Bacc(None, target_bir_lowering=False, debug=False)

    # xT: [granule, partition, k-in-granule, batch]; W: [n, partition, k, nt]
    # cluster/styles arrive duplicated: rows 64-127 = rows 0-63 (row packing).
    xT_d = nc.declare_dram_parameter("xT", [KT // XG, P, XG, MB], bf16, isOutput=False)
    clT_d = nc.declare_dram_parameter("clusterT", [P, MB], bf16, isOutput=False)
    w_d = nc.declare_dram_parameter("weight", [NTS, P, KC, NT], bf16, isOutput=False)
    w8_d = nc.declare_dram_parameter("weight8", [NTS, P, NPR, 2, NT], mybir.dt.float8e4, isOutput=False)
    sL_d = nc.declare_dram_parameter("style_L", [P, DIN], bf16, isOutput=False)
    sR_d = nc.declare_dram_parameter("style_R", [P, DOUT], bf16, isOutput=False)
    out_d = nc.declare_dram_parameter("out", [MB, DOUT], f32, isOutput=True)

    H = NCL  # 64: row-pack halves
    SLW = SLG * P  # cols per sL granule

    with tile.TileContext(nc) as tc:
        with (
            tc.tile_pool(name="const", bufs=1) as const_pool,
            tc.tile_pool(name="atp", bufs=1) as at_pool,
            tc.tile_pool(name="wp", bufs=2) as w_pool,
            tc.tile_pool(name="evp", bufs=3) as ev_pool,
            # PSUM budget (8 banks): py 6 x [128,512] accumulators + pl
            # 1 x [128,1024] fp32 (2 banks) = 8. tmpR pairs borrow pl.
            tc.tile_pool(name="pyp", bufs=6, space="PSUM") as py_pool,
            tc.tile_pool(name="plp", bufs=1, space="PSUM") as pl_pool,
        ):
            # ---- consts that gate the first PE op: tiny, first in queue ----
            clT = const_pool.tile([P, MB], bf16, name="clT")
            nc.sync.dma_start(clT[:, 0:NT], clT_d[:, 0:NT])
            nc.sync.dma_start(clT[:, NT:MB], clT_d[:, NT:MB])
            sL = const_pool.tile([P, DIN], bf16, name="sL")
            nc.sync.dma_start(sL[:, 0:SLW], sL_d[:, 0:SLW])
            sR = const_pool.tile([P, DOUT], bf16, name="sR")

            # W(n=0) granule tiles: independent tiles so their DMAs neither
            # serialize on a shared write-sem nor gate the first MMs.
            NW0G = (KC + W0G - 1) // W0G
            w0g = [
                const_pool.tile([P, min(W0G, KC - j * W0G), NT], bf16, name=f"w0g{j}")
                for j in range(NW0G)
            ]

            def w0slice(k):
                return w0g[k // W0G][:, k % W0G, :]

            f8 = mybir.dt.float8e4
            # fp8 aT pair tiles (pair jp holds k = KC+2jp, KC+2jp+1)
            at8 = [
                const_pool.tile([P, 2, MB], f8, name=f"at8_{jp}")
                for jp in range(NPR)
            ]
            w80 = const_pool.tile([P, NPR, 2, NT], f8, name="w80")

            XG_ORDER = [k // XG for k in PROC_KS[::2]]

            # prologue DMA emission schedule keyed by step index (sync queue
            # order == consumption order; xg runs 2-3 granules ahead)
            def emit_loads(k):
                if k == 0:
                    xg_dma(XG_ORDER[0])
                    nc.sync.dma_start(w0g[0][:], w_d[0, :, 0:W0G, :])

                    nc.sync.dma_start(sR[:], sR_d[:])
                    xg_dma(XG_ORDER[1])
                    xg_dma(XG_ORDER[2])
                    nc.sync.dma_start(w0g[1][:], w_d[0, :, W0G:2 * W0G, :])
                elif k % 2 == 0:
                    gi = k // 2 + 2
                    if gi < KT // XG:
                        xg_dma(XG_ORDER[gi])
                    if k % 4 == 0:
                        j = k // 4 + 1
                        if j < NW0G:
                            nc.sync.dma_start(
                                w0g[j][:],
                                w_d[0, :, j * W0G:min((j + 1) * W0G, KC), :],
                            )
                    if k == 18:
                        nc.sync.dma_start(w80[:], w8_d[0])
                    if k == 2:
                        nc.sync.dma_start(
                            sL[:, SLW:2 * SLW], sL_d[:, SLW:2 * SLW]
                        )
                    elif k == 10:
                        nc.sync.dma_start(
                            sL[:, 2 * SLW:3 * SLW], sL_d[:, 2 * SLW:3 * SLW]
                        )
                    elif k == 14:
                        nc.sync.dma_start(
                            sL[:, 3 * SLW:4 * SLW], sL_d[:, 3 * SLW:4 * SLW]
                        )

            # x/aT merged: xT loads land directly in the resident aT granule
            # tiles (no buffer recycling -> xg DMA issues never wait on
            # at-mul consumption, keeping the strict-FIFO HWDGE ring moving);
            # the at-mul then runs in place (monotone DVE stream, WAR-safe).
            xg_tiles = {}

            def xg_dma(g):
                xg = at_pool.tile([P, XG, MB], bf16, name=f"atg{g}", tag=f"atg{g}")
                nc.sync.dma_start(xg[:], xT_d[g])
                xg_tiles[g] = xg

            def at_view(k):
                return xg_tiles[k // XG][:, k % XG, :]

            def tmpr_pair(n, m, psum_src="pl", copy_eng=None):
                """Row-packed pair: tmpR tiles for (m, m+1) at n, staged to SBUF.

                psum_src="pl" borrows the pl pool tile (two banks) -- fine in
                the body where the next pl user is ~14us away. The n0-prologue
                pairs instead take two py slots BEFORE the fused accumulators
                exist, with copies split scalar/vector so the at-mul stream
                is not delayed.
                """
                if psum_src == "py":
                    pra = py_pool.tile([P, NT], f32, name=f"pr{n}_{m}", tag="py")
                    prb = py_pool.tile([P, NT], f32, name=f"pr{n}_{m + 1}", tag="py")
                else:
                    prp = pl_pool.tile([P, MB], f32, name=f"prf{n}_{m}", tag="pl")
                    pra, prb = prp[:, 0:NT], prp[:, NT:MB]
                nc.tensor.matmul(
                    pra[:],
                    clT[:H, m * P:(m + 1) * P],
                    sR[:H, n * NT:(n + 1) * NT],
                    start=True, stop=True, tile_position=(0, 0),
                )
                nc.tensor.matmul(
                    prb[:],
                    clT[H:, (m + 1) * P:(m + 2) * P],
                    sR[H:, n * NT:(n + 1) * NT],
                    start=True, stop=True, tile_position=(H, 0),
                )
                tra = ev_pool.tile([P, NT], bf16, name=f"tr{n}_{m}", tag="tr", bufs=6)
                trb = ev_pool.tile([P, NT], bf16, name=f"tr{n}_{m + 1}", tag="tr", bufs=6)
                eng = copy_eng or nc.vector
                if eng is nc.scalar:
                    eng.copy(out=tra[:], in_=pra[:])
                    eng.copy(out=trb[:], in_=prb[:])
                else:
                    eng.tensor_copy(out=tra[:], in_=pra[:])
                    eng.tensor_copy(out=trb[:], in_=prb[:])
                return tra, trb

            def epilogue(n, m, py, tr, split=False):
                ot = ev_pool.tile([P, NT], f32, name=f"ot{n}_{m}", tag="ot")
                hc = NT // 2
                chunks = 2 if split else 1
                w = NT // chunks
                for c in range(chunks):
                    nc.vector.tensor_mul(
                        out=ot[:, c * w:(c + 1) * w],
                        in0=py[:, c * w:(c + 1) * w],
                        in1=tr[:, c * w:(c + 1) * w],
                    )
                    nc.scalar.dma_start(
                        out_d[m * P:(m + 1) * P,
                              n * NT + c * w:n * NT + (c + 1) * w],
                        ot[:, c * w:(c + 1) * w],
                    )

            # ---- fused prologue: aT production + n0/m0..5 k-outer
            # accumulation, fused MMs emitted one k behind the tmpLT pairs ----
            py_f = []
            tr_f = []

            def fused_mms(k):
                if not py_f:
                    py_f.extend(
                        py_pool.tile([P, NT], f32, name=f"py0_{m}", tag="py")
                        for m in range(FUSED)
                    )
                if k < KC:
                    for m in range(FUSED):
                        nc.tensor.matmul(
                            py_f[m][:],
                            at_view(k)[:, m * P:(m + 1) * P],
                            w0slice(k),
                            start=(k == 0), stop=(k == KC - 1),
                        )
                elif (k - KC) % 2 == 1:
                    jp = (k - KC) // 2
                    for m in range(FUSED):
                        nc.tensor.matmul(
                            py_f[m][:],
                            at8[jp][:, :, m * P:(m + 1) * P],
                            w80[:, jp, :, :],
                            start=False, stop=False,
                            perf_mode=mybir.MatmulPerfMode.DoubleRow,
                        )

            for step, k in enumerate(PROC_KS):
                emit_loads(step)
                # tmpLT: row-packed pair, both batch halves in one pl slot
                pl = pl_pool.tile([P, MB], f32, name=f"pl{k}", tag="pl")
                nc.tensor.matmul(
                    pl[:, 0:NT],
                    sL[:H, k * P:(k + 1) * P],
                    clT[:H, 0:NT],
                    start=True, stop=True, tile_position=(0, 0),
                )
                nc.tensor.matmul(
                    pl[:, NT:MB],
                    sL[H:, k * P:(k + 1) * P],
                    clT[H:, NT:MB],
                    start=True, stop=True, tile_position=(H, 0),
                )
                if k < KC:
                    nc.vector.tensor_mul(
                        out=at_view(k), in0=at_view(k), in1=pl[:]
                    )
                else:
                    nc.vector.tensor_mul(
                        out=at8[(k - KC) // 2][:, (k - KC) % 2, :],
                        in0=at_view(k), in1=pl[:],
                    )
                if step == 1:
                    # n0 tmpR for the fused m tiles: early, gated only on
                    # clT+sR, psum from the not-yet-allocated accumulator
                    # banks, copies on the otherwise-idle scalar engine
                    # (vector for the last pair so py_f[4:5] free in time).
                    tr_f += tmpr_pair(0, 0, psum_src="py", copy_eng=nc.scalar)
                    tr_f += tmpr_pair(0, 2, psum_src="py", copy_eng=nc.scalar)
                    tr_f += tmpr_pair(0, 4, psum_src="py", copy_eng=nc.vector)
                if step > 0:
                    fused_mms(PROC_KS[step - 1])
            fused_mms(PROC_KS[-1])
            for m in range(FUSED):
                epilogue(0, m, py_f[m], tr_f[m])

            # ---- standard m-pair body: two 32-MM groups with the packed tmpR
            # pair injected mid-group (the deep MM pipeline hides its
            # LDWEIGHTS; at a group boundary it costs a full extra slot) ----
            def body_pair(n, m, wsl, w8n):
                tra = trb = None
                for mm in (m, m + 1):
                    py = py_pool.tile([P, NT], f32, name=f"py{n}_{mm}", tag="py")
                    for step, k in enumerate(PROC_KS):
                        if k < KC:
                            nc.tensor.matmul(
                                py[:],
                                at_view(k)[:, mm * P:(mm + 1) * P],
                                wsl(k),
                                start=(k == 0), stop=(k == KC - 1),
                            )
                        elif (k - KC) % 2 == 1:
                            jp = (k - KC) // 2
                            nc.tensor.matmul(
                                py[:],
                                at8[jp][:, :, mm * P:(mm + 1) * P],
                                w80n[jp] if w8n is None else w8n[:, jp, :, :],
                                start=False, stop=False,
                                perf_mode=mybir.MatmulPerfMode.DoubleRow,
                            )
                        if mm == m and step == KT // 2:
                            tra, trb = tmpr_pair(n, m)
                    epilogue(n, mm, py, tra if mm == m else trb,
                             split=(n == NTS - 1 and mm == MT - 1))

            w80n = [w80[:, jp, :, :] for jp in range(NPR)]

            # rest of n=0 (w1 prefetch queues behind the prologue loads)
            w1 = w_pool.tile([P, KC, NT], bf16, name="w1", tag="wbig")
            nc.sync.dma_start(w1[:], w_d[1])
            w81 = ev_pool.tile([P, NPR, 2, NT], f8, name="w81", tag="w8", bufs=2)
            nc.sync.dma_start(w81[:], w8_d[1])
            for m in range(FUSED, MT, 2):
                body_pair(0, m, w0slice, None)
            # n = 1..7
            wn, w8n = w1, w81
            for n in range(1, NTS):
                if n + 1 < NTS:
                    wnext = w_pool.tile([P, KC, NT], bf16, name=f"w{n+1}", tag="wbig")
                    nc.sync.dma_start(wnext[:], w_d[n + 1])
                    w8next = ev_pool.tile([P, NPR, 2, NT], f8, name=f"w8{n+1}", tag="w8", bufs=2)
                    nc.sync.dma_start(w8next[:], w8_d[n + 1])
                wcur, w8cur = wn, w8n
                for m in range(0, MT, 2):
                    body_pair(n, m, lambda k, w=wcur: w[:, k, :], w8cur)
                if n + 1 < NTS:
                    wn, w8n = wnext, w8next

    nc.finalize()
    return nc


def _get_program():
    if "nc" not in _CACHE:
        _CACHE["nc"] = _build_program()
    return _CACHE["nc"]


def kernel(x, cluster, weight, style_L, style_R):
    import os

    # The NTFF trace path needs an antenv hook this container lacks; never
    # let a stray BASS_TRACE env take the run down that path.
    os.environ.setdefault("BASS_NEVER_TRACE", "1")
    from concourse.bass_utils import run_bass_kernel_spmd

    nc = _get_program()
    bf16 = ml_dtypes.bfloat16

    # W: [din, dout] -> [n, p, k, nt] partition-major for contiguous DMA.
    # All weights are scaled x64 (exact in bf16; brings the fp8 tail into
    # e4m3 range) and style_R is scaled 1/64 to compensate.
    wsc = np.asarray(weight, dtype=np.float32) * 64.0
    w_full = wsc.reshape(KT, P, NTS, NT)
    w_r = np.ascontiguousarray(
        w_full[:KC].astype(bf16).transpose(2, 1, 0, 3)
    )
    # fp8 tail: [NTS, P, pair, half, NT] e4m3 pairs for DoubleRow
    w_8 = np.ascontiguousarray(
        w_full[KC:].reshape(NPR, 2, P, NTS, NT)
        .transpose(3, 2, 0, 1, 4)
    ).astype(ml_dtypes.float8_e4m3fn)
    # styles/cluster duplicated across both 64-row halves for row packing
    sL1 = np.asarray(style_L, dtype=np.float32).astype(bf16)
    sR1 = (np.asarray(style_R, dtype=np.float32) / 64.0).astype(bf16)
    sL = np.ascontiguousarray(np.vstack([sL1, sL1]))
    sR = np.ascontiguousarray(np.vstack([sR1, sR1]))

    in_maps = []
    for c in range(NCORES):
        xs = np.asarray(x[c * MB:(c + 1) * MB], dtype=np.float32)
        xT = np.ascontiguousarray(xs.T).astype(bf16)          # [DIN, MB]
        # [din, mb] -> [granule, p, k-in-granule, mb]
        xT_r = np.ascontiguousarray(
            xT.reshape(KT // XG, XG, P, MB).transpose(0, 2, 1, 3)
        )
        clT1 = np.ascontiguousarray(
            np.asarray(cluster[c * MB:(c + 1) * MB], dtype=np.float32).T
        ).astype(bf16)
        clT = np.ascontiguousarray(np.vstack([clT1, clT1]))
        in_maps.append(
            {"xT": xT_r, "clusterT": clT, "weight": w_r, "weight8": w_8,
             "style_L": sL, "style_R": sR}
        )

    res = run_bass_kernel_spmd(nc, in_maps, list(range(NCORES)))
    LAST["results"] = res
    LAST["in_maps"] = in_maps
    out = np.concatenate(
        [np.asarray(res.results[c]["out"], dtype=np.float32) for c in range(NCORES)],
        axis=0,
    )
    return out


# revision 13
# speedup vs baseline: 1.0080x; 1.0080x over previous
# Bass/Tile TRN2 kernel for nn_Conv1D_style: out = ((x * (cluster@style_L)) @ weight) * (cluster@style_R)
#
# Sharding: data-parallel over the batch dim. Each of the 8 cores gets a
# 1024-row slice of x/cluster and a full (replicated) weight/style_L/style_R.
#
# Per-core plan (M=1024 batch, K=4096 din, N=4096 dout), all matmuls bf16
# with fp32 PSUM accumulation:
#   aT[k] = xT[k] * (style_L[:, kslice].T @ clusterT)  -> bf16, SBUF-resident.
#   y[m,n] = sum_k aT[k][:, mslice].T @ W[k, nslice]   (32 accumulating MMs)
#   out[m,n] = y[m,n] * (clusterT[:, mslice].T @ style_R[:, nslice])
#
# Schedule: the aT production is software-pipelined with the n=0
# accumulation for m0-5 (6 PSUM accumulators, k-outer): per k the PE runs
# [tmpLT pair k][6 fused MMs for k-1], so the vector at-mul for k (1.29us)
# hides under the fused MM group (1.30us) with pl bufs=1 acting as the
# pipeline interlock. All loads stream on the Sync HWDGE queue in exact
# consumption order with fine granules (xT in 2-k-tile granules, W(n=0) in
# 4-k-tile granule tiles, style_L in 8-k-tile granules) so the first PE op
# is gated on ~0.5 MiB, not 13 MiB; w1/w2 prefetch queues strictly after
# the prologue bytes. Output stores (and nothing else) issue on the
# Scalar/Activation queue so stores never delay loads.
#
# The K=64 style matmuls (tmpLT/tmpR) are row-packed two at a time via
# tile_position into the upper/lower 64 PE rows: the host ships
# cluster/style operands duplicated across partitions 0-63 and 64-127, and
# each packed pair costs one ~320ns slot instead of two.

import numpy as np
import ml_dtypes

B, DIN, DOUT, NCL = 8192, 4096, 4096, 64
NCORES = 8
MB = B // NCORES          # batch rows per core
P = 128
NT = 512                  # n tile (dout cols per matmul)
KT = DIN // P             # 32 k tiles
MT = MB // P              # 8 m tiles
NTS = DOUT // NT          # 8 n tiles
FUSED = 6                 # m tiles of n=0 accumulated during the aT prologue
XG = 2                    # k tiles per xT DMA granule (16 granules)
W0G = 4                   # k tiles per W(n=0) granule tile (8 granules)
SLG = 8                   # k tiles per style_L granule (4 granules)
FP8K = 6                  # k tiles 26..31 run as fp8e4 DoubleRow pairs
KC = KT - FP8K            # bf16 k tiles (0..KC-1)
NPR = FP8K // 2           # DoubleRow pair count (3)
# processing order: fp8 pairs (PE-light vs the at-mul window) interleaved
# between PE-heavy bf16 steps so the PE work queue never drains
PROC_KS = (list(range(0, 20))
           + [26, 27, 20, 21, 28, 29, 22, 23, 30, 31, 24, 25])

_CACHE = {}
LAST = {}                 # exposes the most recent BassKernelResults for test harnesses


def _build_program():
    import concourse.bacc as bacc
    import concourse.mybir as mybir
    import concourse.tile as tile

    bf16 = mybir.dt.bfloat16
    f32 = mybir.dt.float32

    nc = bacc.Bacc(None, target_bir_lowering=False, debug=False)

    # xT: [granule, partition, k-in-granule, batch]; W: [n, partition, k, nt]
    # cluster/styles arrive duplicated: rows 64-127 = rows 0-63 (row packing).
    xT_d = nc.declare_dram_parameter("xT", [KT // XG, P, XG, MB], bf16, isOutput=False)
    clT_d = nc.declare_dram_parameter("clusterT", [P, MB], bf16, isOutput=False)
    w_d = nc.declare_dram_parameter("weight", [NTS, P, KC, NT], bf16, isOutput=False)
    w8_d = nc.declare_dram_parameter("weight8", [NTS, P, NPR, 2, NT], mybir.dt.float8e4, isOutput=False)
    sL_d = nc.declare_dram_parameter("style_L", [P, DIN], bf16, isOutput=False)
    sR_d = nc.declare_dram_parameter("style_R", [P, DOUT], bf16, isOutput=False)
    out_d = nc.declare_dram_parameter("out", [MB, DOUT], f32, isOutput=True)

    H = NCL  # 64: row-pack halves
    SLW = SLG * P  # cols per sL granule

    with tile.TileContext(nc) as tc:
        with (
            tc.tile_pool(name="const", bufs=1) as const_pool,
            tc.tile_pool(name="atp", bufs=1) as at_pool,
            tc.tile_pool(name="wp", bufs=2) as w_pool,
            tc.tile_pool(name="evp", bufs=3) as ev_pool,
            # PSUM budget (8 banks): py 6 x [128,512] accumulators + pl
            # 1 x [128,1024] fp32 (2 banks) = 8. tmpR pairs borrow pl.
            tc.tile_pool(name="pyp", bufs=6, space="PSUM") as py_pool,
            tc.tile_pool(name="plp", bufs=1, space="PSUM") as pl_pool,
        ):
            # ---- consts that gate the first PE op: tiny, first in queue ----
            clT = const_pool.tile([P, MB], bf16, name="clT")
            nc.sync.dma_start(clT[:, 0:NT], clT_d[:, 0:NT])
            nc.sync.dma_start(clT[:, NT:MB], clT_d[:, NT:MB])
            sL = const_pool.tile([P, DIN], bf16, name="sL")
            nc.sync.dma_start(sL[:, 0:SLW], sL_d[:, 0:SLW])
            sR = const_pool.tile([P, DOUT], bf16, name="sR")

            # W(n=0) granule tiles: independent tiles so their DMAs neither
            # serialize on a shared write-sem nor gate the first MMs.
            NW0G = (KC + W0G - 1) // W0G
            w0g = [
                const_pool.tile([P, min(W0G, KC - j * W0G), NT], bf16, name=f"w0g{j}")
                for j in range(NW0G)
            ]

            def w0slice(k):
                return w0g[k // W0G][:, k % W0G, :]

            f8 = mybir.dt.float8e4
            # fp8 aT pair tiles (pair jp holds k = KC+2jp, KC+2jp+1)
            at8 = [
                const_pool.tile([P, 2, MB], f8, name=f"at8_{jp}")
                for jp in range(NPR)
            ]
            w80 = const_pool.tile([P, NPR, 2, NT], f8, name="w80")

            XG_ORDER = [k // XG for k in PROC_KS[::2]]

            # prologue DMA emission schedule keyed by step index (sync queue
            # order == consumption order; xg runs 2-3 granules ahead)
            def emit_loads(k):
                if k == 0:
                    xg_dma(XG_ORDER[0])
                    nc.sync.dma_start(w0g[0][:], w_d[0, :, 0:W0G, :])

                    nc.sync.dma_start(sR[:], sR_d[:])
                    xg_dma(XG_ORDER[1])
                    xg_dma(XG_ORDER[2])
                    nc.sync.dma_start(w0g[1][:], w_d[0, :, W0G:2 * W0G, :])
                elif k % 2 == 0:
                    gi = k // 2 + 2
                    if gi < KT // XG:
                        xg_dma(XG_ORDER[gi])
                    if k % 4 == 0:
                        j = k // 4 + 1
                        if j < NW0G:
                            nc.sync.dma_start(
                                w0g[j][:],
                                w_d[0, :, j * W0G:min((j + 1) * W0G, KC), :],
                            )
                    if k == 18:
                        nc.sync.dma_start(w80[:], w8_d[0])
                    if k == 2:
                        nc.sync.dma_start(
                            sL[:, SLW:2 * SLW], sL_d[:, SLW:2 * SLW]
                        )
                    elif k == 10:
                        nc.sync.dma_start(
                            sL[:, 2 * SLW:3 * SLW], sL_d[:, 2 * SLW:3 * SLW]
                        )
                    elif k == 14:
                        nc.sync.dma_start(
                            sL[:, 3 * SLW:4 * SLW], sL_d[:, 3 * SLW:4 * SLW]
                        )

            # x/aT merged: xT loads land directly in the resident aT granule
            # tiles (no buffer recycling -> xg DMA issues never wait on
            # at-mul consumption, keeping the strict-FIFO HWDGE ring moving);
            # the at-mul then runs in place (monotone DVE stream, WAR-safe).
            xg_tiles = {}

            def xg_dma(g):
                xg = at_pool.tile([P, XG, MB], bf16, name=f"atg{g}", tag=f"atg{g}")
                nc.sync.dma_start(xg[:], xT_d[g])
                xg_tiles[g] = xg

            def at_view(k):
                return xg_tiles[k // XG][:, k % XG, :]

            def tmpr_pair(n, m, psum_src="pl", copy_eng=None):
                """Row-packed pair: tmpR tiles for (m, m+1) at n, staged to SBUF.

                psum_src="pl" borrows the pl pool tile (two banks) -- fine in
                the body where the next pl user is ~14us away. The n0-prologue
                pairs instead take two py slots BEFORE the fused accumulators
                exist, with copies split scalar/vector so the at-mul stream
                is not delayed.
                """
                if psum_src == "py":
                    pra = py_pool.tile([P, NT], f32, name=f"pr{n}_{m}", tag="py")
                    prb = py_pool.tile([P, NT], f32, name=f"pr{n}_{m + 1}", tag="py")
                else:
                    prp = pl_pool.tile([P, MB], f32, name=f"prf{n}_{m}", tag="pl")
                    pra, prb = prp[:, 0:NT], prp[:, NT:MB]
                nc.tensor.matmul(
                    pra[:],
                    clT[:H, m * P:(m + 1) * P],
                    sR[:H, n * NT:(n + 1) * NT],
                    start=True, stop=True, tile_position=(0, 0),
                )
                nc.tensor.matmul(
                    prb[:],
                    clT[H:, (m + 1) * P:(m + 2) * P],
                    sR[H:, n * NT:(n + 1) * NT],
                    start=True, stop=True, tile_position=(H, 0),
                )
                tra = ev_pool.tile([P, NT], bf16, name=f"tr{n}_{m}", tag="tr", bufs=6)
                trb = ev_pool.tile([P, NT], bf16, name=f"tr{n}_{m + 1}", tag="tr", bufs=6)
                eng = copy_eng or nc.vector
                if eng is nc.scalar:
                    eng.copy(out=tra[:], in_=pra[:])
                    eng.copy(out=trb[:], in_=prb[:])
                else:
                    eng.tensor_copy(out=tra[:], in_=pra[:])
                    eng.tensor_copy(out=trb[:], in_=prb[:])
                return tra, trb

            def epilogue(n, m, py, tr, split=False):
                ot = ev_pool.tile([P, NT], f32, name=f"ot{n}_{m}", tag="ot")
                hc = NT // 2
                chunks = 2 if split else 1
                w = NT // chunks
                for c in range(chunks):
                    nc.vector.tensor_mul(
                        out=ot[:, c * w:(c + 1) * w],
                        in0=py[:, c * w:(c + 1) * w],
                        in1=tr[:, c * w:(c + 1) * w],
                    )
                    nc.scalar.dma_start(
                        out_d[m * P:(m + 1) * P,
                              n * NT + c * w:n * NT + (c + 1) * w],
                        ot[:, c * w:(c + 1) * w],
                    )

            # ---- fused prologue: aT production + n0/m0..5 k-outer
            # accumulation, fused MMs emitted one k behind the tmpLT pairs ----
            py_f = []
            tr_f = []

            def fused_mms(k):
                if not py_f:
                    py_f.extend(
                        py_pool.tile([P, NT], f32, name=f"py0_{m}", tag="py")
                        for m in range(FUSED)
                    )
                if k < KC:
                    for m in range(FUSED):
                        nc.tensor.matmul(
                            py_f[m][:],
                            at_view(k)[:, m * P:(m + 1) * P],
                            w0slice(k),
                            start=(k == 0), stop=(k == KC - 1),
                        )
                elif (k - KC) % 2 == 1:
                    jp = (k - KC) // 2
                    for m in range(FUSED):
                        nc.tensor.matmul(
                            py_f[m][:],
                            at8[jp][:, :, m * P:(m + 1) * P],
                            w80[:, jp, :, :],
                            start=False, stop=False,
                            perf_mode=mybir.MatmulPerfMode.DoubleRow,
                        )

            for step, k in enumerate(PROC_KS):
                emit_loads(step)
                # tmpLT: row-packed pair, both batch halves in one pl slot
                pl = pl_pool.tile([P, MB], f32, name=f"pl{k}", tag="pl")
                nc.tensor.matmul(
                    pl[:, 0:NT],
                    sL[:H, k * P:(k + 1) * P],
                    clT[:H, 0:NT],
                    start=True, stop=True, tile_position=(0, 0),
                )
                nc.tensor.matmul(
                    pl[:, NT:MB],
                    sL[H:, k * P:(k + 1) * P],
                    clT[H:, NT:MB],
                    start=True, stop=True, tile_position=(H, 0),
                )
                if k < KC:
                    nc.vector.tensor_mul(
                        out=at_view(k), in0=at_view(k), in1=pl[:]
                    )
                else:
                    nc.vector.tensor_mul(
                        out=at8[(k - KC) // 2][:, (k - KC) % 2, :],
                        in0=at_view(k), in1=pl[:],
                    )
                if step == 1:
                    # n0 tmpR for the fused m tiles: early, gated only on
                    # clT+sR, psum from the not-yet-allocated accumulator
                    # banks, copies on the otherwise-idle scalar engine
                    # (vector for the last pair so py_f[4:5] free in time).
                    tr_f += tmpr_pair(0, 0, psum_src="py", copy_eng=nc.scalar)
                    tr_f += tmpr_pair(0, 2, psum_src="py", copy_eng=nc.scalar)
                    tr_f += tmpr_pair(0, 4, psum_src="py", copy_eng=nc.vector)
                if step > 0:
                    fused_mms(PROC_KS[step - 1])
            fused_mms(PROC_KS[-1])
            for m in range(FUSED):
                epilogue(0, m, py_f[m], tr_f[m])

            # ---- standard m-pair body: two 32-MM groups with the packed tmpR
            # pair injected mid-group (the deep MM pipeline hides its
            # LDWEIGHTS; at a group boundary it costs a full extra slot) ----
            def body_pair(n, m, wsl, w8n):
                tra = trb = None
                for mm in (m, m + 1):
                    py = py_pool.tile([P, NT], f32, name=f"py{n}_{mm}", tag="py")
                    for k in range(KC):
                        nc.tensor.matmul(
                            py[:],
                            at_view(k)[:, mm * P:(mm + 1) * P],
                            wsl(k),
                            start=(k == 0), stop=False,
                        )
                        if mm == m and k == KT // 2:
                            tra, trb = tmpr_pair(n, m)
                    for jp in range(NPR):
                        nc.tensor.matmul(
                            py[:],
                            at8[jp][:, :, mm * P:(mm + 1) * P],
                            w80n[jp] if w8n is None else w8n[:, jp, :, :],
                            start=False, stop=(jp == NPR - 1),
                            perf_mode=mybir.MatmulPerfMode.DoubleRow,
                        )
                    epilogue(n, mm, py, tra if mm == m else trb,
                             split=(n == NTS - 1 and mm == MT - 1))

            w80n = [w80[:, jp, :, :] for jp in range(NPR)]

            # rest of n=0 (w1 prefetch queues behind the prologue loads)
            w1 = w_pool.tile([P, KC, NT], bf16, name="w1", tag="wbig")
            nc.sync.dma_start(w1[:], w_d[1])
            w81 = ev_pool.tile([P, NPR, 2, NT], f8, name="w81", tag="w8", bufs=2)
            nc.sync.dma_start(w81[:], w8_d[1])
            for m in range(FUSED, MT, 2):
                body_pair(0, m, w0slice, None)
            # n = 1..7
            wn, w8n = w1, w81
            for n in range(1, NTS):
                if n + 1 < NTS:
                    wnext = w_pool.tile([P, KC, NT], bf16, name=f"w{n+1}", tag="wbig")
                    nc.sync.dma_start(wnext[:], w_d[n + 1])
                    w8next = ev_pool.tile([P, NPR, 2, NT], f8, name=f"w8{n+1}", tag="w8", bufs=2)
                    nc.sync.dma_start(w8next[:], w8_d[n + 1])
                wcur, w8cur = wn, w8n
                for m in range(0, MT, 2):
                    body_pair(n, m, lambda k, w=wcur: w[:, k, :], w8cur)
                if n + 1 < NTS:
                    wn, w8n = wnext, w8next

    nc.finalize()
    return nc


def _get_program():
    if "nc" not in _CACHE:
        _CACHE["nc"] = _build_program()
    return _CACHE["nc"]


def kernel(x, cluster, weight, style_L, style_R):
    import os

    # The NTFF trace path needs an antenv hook this container lacks; never
    # let a stray BASS_TRACE env take the run down that path.
    os.environ.setdefault("BASS_NEVER_TRACE", "1")
    from concourse.bass_utils import run_bass_kernel_spmd

    nc = _get_program()
    bf16 = ml_dtypes.bfloat16

    # W: [din, dout] -> [n, p, k, nt] partition-major for contiguous DMA.
    # All weights are scaled x64 (exact in bf16; brings the fp8 tail into
    # e4m3 range) and style_R is scaled 1/64 to compensate.
    wsc = np.asarray(weight, dtype=np.float32) * 64.0
    w_full = wsc.reshape(KT, P, NTS, NT)
    w_r = np.ascontiguousarray(
        w_full[:KC].astype(bf16).transpose(2, 1, 0, 3)
    )
    # fp8 tail: [NTS, P, pair, half, NT] e4m3 pairs for DoubleRow
    w_8 = np.ascontiguousarray(
        w_full[KC:].reshape(NPR, 2, P, NTS, NT)
        .transpose(3, 2, 0, 1, 4)
    ).astype(ml_dtypes.float8_e4m3fn)
    # styles/cluster duplicated across both 64-row halves for row packing
    sL1 = np.asarray(style_L, dtype=np.float32).astype(bf16)
    sR1 = (np.asarray(style_R, dtype=np.float32) / 64.0).astype(bf16)
    sL = np.ascontiguousarray(np.vstack([sL1, sL1]))
    sR = np.ascontiguousarray(np.vstack([sR1, sR1]))

    in_maps = []
    for c in range(NCORES):
        xs = np.asarray(x[c * MB:(c + 1) * MB], dtype=np.float32)
        xT = np.ascontiguousarray(xs.T).astype(bf16)          # [DIN, MB]
        # [din, mb] -> [granule, p, k-in-granule, mb]
        xT_r = np.ascontiguousarray(
            xT.reshape(KT // XG, XG, P, MB).transpose(0, 2, 1, 3)
        )
        clT1 = np.ascontiguousarray(
            np.asarray(cluster[c * MB:(c + 1) * MB], dtype=np.float32).T
        ).astype(bf16)
        clT = np.ascontiguousarray(np.vstack([clT1, clT1]))
        in_maps.append(
            {"xT": xT_r, "clusterT": clT, "weight": w_r, "weight8": w_8,
             "style_L": sL, "style_R": sR}
        )

    res = run_bass_kernel_spmd(nc, in_maps, list(range(NCORES)))
    LAST["results"] = res
    LAST["in_maps"] = in_maps
    out = np.concatenate(
        [np.asarray(res.results[c]["out"], dtype=np.float32) for c in range(NCORES)],
        axis=0,
    )
    return out


# revision 14
# speedup vs baseline: 1.0088x; 1.0007x over previous
# Bass/Tile TRN2 kernel for nn_Conv1D_style: out = ((x * (cluster@style_L)) @ weight) * (cluster@style_R)
#
# Sharding: data-parallel over the batch dim. Each of the 8 cores gets a
# 1024-row slice of x/cluster and a full (replicated) weight/style_L/style_R.
#
# Per-core plan (M=1024 batch, K=4096 din, N=4096 dout), all matmuls bf16
# with fp32 PSUM accumulation:
#   aT[k] = xT[k] * (style_L[:, kslice].T @ clusterT)  -> bf16, SBUF-resident.
#   y[m,n] = sum_k aT[k][:, mslice].T @ W[k, nslice]   (32 accumulating MMs)
#   out[m,n] = y[m,n] * (clusterT[:, mslice].T @ style_R[:, nslice])
#
# Schedule: the aT production is software-pipelined with the n=0
# accumulation for m0-5 (6 PSUM accumulators, k-outer): per k the PE runs
# [tmpLT pair k][6 fused MMs for k-1], so the vector at-mul for k (1.29us)
# hides under the fused MM group (1.30us) with pl bufs=1 acting as the
# pipeline interlock. All loads stream on the Sync HWDGE queue in exact
# consumption order with fine granules (xT in 2-k-tile granules, W(n=0) in
# 4-k-tile granule tiles, style_L in 8-k-tile granules) so the first PE op
# is gated on ~0.5 MiB, not 13 MiB; w1/w2 prefetch queues strictly after
# the prologue bytes. Output stores (and nothing else) issue on the
# Scalar/Activation queue so stores never delay loads.
#
# The K=64 style matmuls (tmpLT/tmpR) are row-packed two at a time via
# tile_position into the upper/lower 64 PE rows: the host ships
# cluster/style operands duplicated across partitions 0-63 and 64-127, and
# each packed pair costs one ~320ns slot instead of two.

import numpy as np
import ml_dtypes

B, DIN, DOUT, NCL = 8192, 4096, 4096, 64
NCORES = 8
MB = B // NCORES          # batch rows per core
P = 128
NT = 512                  # n tile (dout cols per matmul)
KT = DIN // P             # 32 k tiles
MT = MB // P              # 8 m tiles
NTS = DOUT // NT          # 8 n tiles
FUSED = 6                 # m tiles of n=0 accumulated during the aT prologue
XG = 2                    # k tiles per xT DMA granule (16 granules)
W0G = 4                   # k tiles per W(n=0) granule tile (8 granules)
SLG = 8                   # k tiles per style_L granule (4 granules)
FP8K = 6                  # trailing k tiles run as fp8e4 DoubleRow pairs
KC = KT - FP8K            # bf16 k tiles (0..KC-1)
NPR = FP8K // 2           # DoubleRow pair count (3)

_CACHE = {}
LAST = {}                 # exposes the most recent BassKernelResults for test harnesses


def _build_program():
    import concourse.bacc as bacc
    import concourse.mybir as mybir
    import concourse.tile as tile

    bf16 = mybir.dt.bfloat16
    f32 = mybir.dt.float32

    nc = bacc.Bacc(None, target_bir_lowering=False, debug=False)

    # xT: [granule, partition, k-in-granule, batch]; W: [n, partition, k, nt]
    # cluster/styles arrive duplicated: rows 64-127 = rows 0-63 (row packing).
    xT_d = nc.declare_dram_parameter("xT", [KT // XG, P, XG, MB], bf16, isOutput=False)
    clT_d = nc.declare_dram_parameter("clusterT", [P, MB], bf16, isOutput=False)
    w_d = nc.declare_dram_parameter("weight", [NTS, P, KC, NT], bf16, isOutput=False)
    w8_d = nc.declare_dram_parameter("weight8", [NTS, P, NPR, 2, NT], mybir.dt.float8e4, isOutput=False)
    sL_d = nc.declare_dram_parameter("style_L", [P, DIN], bf16, isOutput=False)
    sR_d = nc.declare_dram_parameter("style_R", [P, DOUT], bf16, isOutput=False)
    out_d = nc.declare_dram_parameter("out", [MB, DOUT], f32, isOutput=True)

    H = NCL  # 64: row-pack halves
    SLW = SLG * P  # cols per sL granule

    with tile.TileContext(nc) as tc:
        with (
            tc.tile_pool(name="const", bufs=1) as const_pool,
            tc.tile_pool(name="atp", bufs=1) as at_pool,
            tc.tile_pool(name="wp", bufs=2) as w_pool,
            tc.tile_pool(name="evp", bufs=3) as ev_pool,
            # PSUM budget (8 banks): py 6 x [128,512] accumulators + pl
            # 1 x [128,1024] fp32 (2 banks) = 8. tmpR pairs borrow pl.
            tc.tile_pool(name="pyp", bufs=6, space="PSUM") as py_pool,
            tc.tile_pool(name="plp", bufs=1, space="PSUM") as pl_pool,
        ):
            # ---- consts that gate the first PE op: tiny, first in queue ----
            clT = const_pool.tile([P, MB], bf16, name="clT")
            nc.sync.dma_start(clT[:], clT_d[:])
            sL = const_pool.tile([P, DIN], bf16, name="sL")
            nc.sync.dma_start(sL[:, 0:SLW], sL_d[:, 0:SLW])
            sR = const_pool.tile([P, DOUT], bf16, name="sR")

            # W(n=0) granule tiles: independent tiles so their DMAs neither
            # serialize on a shared write-sem nor gate the first MMs.
            NW0G = (KC + W0G - 1) // W0G
            w0g = [
                const_pool.tile([P, min(W0G, KC - j * W0G), NT], bf16, name=f"w0g{j}")
                for j in range(NW0G)
            ]

            def w0slice(k):
                return w0g[k // W0G][:, k % W0G, :]

            f8 = mybir.dt.float8e4
            # fp8 aT pair tiles (pair jp holds k = KC+2jp, KC+2jp+1)
            at8 = [
                const_pool.tile([P, 2, MB], f8, name=f"at8_{jp}")
                for jp in range(NPR)
            ]
            w80 = const_pool.tile([P, NPR, 2, NT], f8, name="w80")

            # prologue DMA emission schedule keyed by k (sync queue order ==
            # consumption order; xg runs 2-3 granules ahead of the at-muls)
            def emit_loads(k):
                if k == 0:
                    xg_dma(0)
                    nc.sync.dma_start(w0g[0][:], w_d[0, :, 0:W0G, :])

                    nc.sync.dma_start(sR[:], sR_d[:])
                    xg_dma(1)
                    xg_dma(2)
                    nc.sync.dma_start(w0g[1][:], w_d[0, :, W0G:2 * W0G, :])
                elif k % 2 == 0:
                    g = k // 2 + 2
                    if g < KT // XG:
                        xg_dma(g)
                    if k % 4 == 0:
                        j = k // 4 + 1
                        if j < NW0G:
                            nc.sync.dma_start(
                                w0g[j][:],
                                w_d[0, :, j * W0G:min((j + 1) * W0G, KC), :],
                            )
                    if k == 18:
                        nc.sync.dma_start(w80[:], w8_d[0])
                    if k == 2:
                        nc.sync.dma_start(
                            sL[:, SLW:2 * SLW], sL_d[:, SLW:2 * SLW]
                        )
                    elif k == 10:
                        nc.sync.dma_start(
                            sL[:, 2 * SLW:3 * SLW], sL_d[:, 2 * SLW:3 * SLW]
                        )
                    elif k == 14:
                        nc.sync.dma_start(
                            sL[:, 3 * SLW:4 * SLW], sL_d[:, 3 * SLW:4 * SLW]
                        )

            # x/aT merged: xT loads land directly in the resident aT granule
            # tiles (no buffer recycling -> xg DMA issues never wait on
            # at-mul consumption, keeping the strict-FIFO HWDGE ring moving);
            # the at-mul then runs in place (monotone DVE stream, WAR-safe).
            xg_tiles = {}

            def xg_dma(g):
                xg = at_pool.tile([P, XG, MB], bf16, name=f"atg{g}", tag=f"atg{g}")
                nc.sync.dma_start(xg[:], xT_d[g])
                xg_tiles[g] = xg

            def at_view(k):
                return xg_tiles[k // XG][:, k % XG, :]

            def tmpr_pair(n, m, psum_src="pl", copy_eng=None):
                """Row-packed pair: tmpR tiles for (m, m+1) at n, staged to SBUF.

                psum_src="pl" borrows the pl pool tile (two banks) -- fine in
                the body where the next pl user is ~14us away. The n0-prologue
                pairs instead take two py slots BEFORE the fused accumulators
                exist, with copies split scalar/vector so the at-mul stream
                is not delayed.
                """
                if psum_src == "py":
                    pra = py_pool.tile([P, NT], f32, name=f"pr{n}_{m}", tag="py")
                    prb = py_pool.tile([P, NT], f32, name=f"pr{n}_{m + 1}", tag="py")
                else:
                    prp = pl_pool.tile([P, MB], f32, name=f"prf{n}_{m}", tag="pl")
                    pra, prb = prp[:, 0:NT], prp[:, NT:MB]
                nc.tensor.matmul(
                    pra[:],
                    clT[:H, m * P:(m + 1) * P],
                    sR[:H, n * NT:(n + 1) * NT],
                    start=True, stop=True, tile_position=(0, 0),
                )
                nc.tensor.matmul(
                    prb[:],
                    clT[H:, (m + 1) * P:(m + 2) * P],
                    sR[H:, n * NT:(n + 1) * NT],
                    start=True, stop=True, tile_position=(H, 0),
                )
                tra = ev_pool.tile([P, NT], bf16, name=f"tr{n}_{m}", tag="tr", bufs=6)
                trb = ev_pool.tile([P, NT], bf16, name=f"tr{n}_{m + 1}", tag="tr", bufs=6)
                eng = copy_eng or nc.vector
                if eng is nc.scalar:
                    eng.copy(out=tra[:], in_=pra[:])
                    eng.copy(out=trb[:], in_=prb[:])
                else:
                    eng.tensor_copy(out=tra[:], in_=pra[:])
                    eng.tensor_copy(out=trb[:], in_=prb[:])
                return tra, trb

            def epilogue(n, m, py, tr):
                ot = ev_pool.tile([P, NT], f32, name=f"ot{n}_{m}", tag="ot")
                nc.vector.tensor_mul(out=ot[:], in0=py[:], in1=tr[:])
                nc.scalar.dma_start(
                    out_d[m * P:(m + 1) * P, n * NT:(n + 1) * NT], ot[:]
                )

            # ---- fused prologue: aT production + n0/m0..5 k-outer
            # accumulation, fused MMs emitted one k behind the tmpLT pairs ----
            py_f = []
            tr_f = []

            def fused_mms(k):
                if not py_f:
                    py_f.extend(
                        py_pool.tile([P, NT], f32, name=f"py0_{m}", tag="py")
                        for m in range(FUSED)
                    )
                if k < KC:
                    for m in range(FUSED):
                        nc.tensor.matmul(
                            py_f[m][:],
                            at_view(k)[:, m * P:(m + 1) * P],
                            w0slice(k),
                            start=(k == 0), stop=False,
                        )
                elif (k - KC) % 2 == 1:
                    jp = (k - KC) // 2
                    for m in range(FUSED):
                        nc.tensor.matmul(
                            py_f[m][:],
                            at8[jp][:, :, m * P:(m + 1) * P],
                            w80[:, jp, :, :],
                            start=False, stop=(jp == NPR - 1),
                            perf_mode=mybir.MatmulPerfMode.DoubleRow,
                        )

            for k in range(KT):
                emit_loads(k)
                # tmpLT: row-packed pair, both batch halves in one pl slot
                pl = pl_pool.tile([P, MB], f32, name=f"pl{k}", tag="pl")
                nc.tensor.matmul(
                    pl[:, 0:NT],
                    sL[:H, k * P:(k + 1) * P],
                    clT[:H, 0:NT],
                    start=True, stop=True, tile_position=(0, 0),
                )
                nc.tensor.matmul(
                    pl[:, NT:MB],
                    sL[H:, k * P:(k + 1) * P],
                    clT[H:, NT:MB],
                    start=True, stop=True, tile_position=(H, 0),
                )
                if k < KC:
                    nc.vector.tensor_mul(
                        out=at_view(k), in0=at_view(k), in1=pl[:]
                    )
                else:
                    nc.vector.tensor_mul(
                        out=at8[(k - KC) // 2][:, (k - KC) % 2, :],
                        in0=at_view(k), in1=pl[:],
                    )
                if k == 1:
                    # n0 tmpR for the fused m tiles: early, gated only on
                    # clT+sR, psum from the not-yet-allocated accumulator
                    # banks, copies on the otherwise-idle scalar engine
                    # (vector for the last pair so py_f[4:5] free in time).
                    tr_f += tmpr_pair(0, 0, psum_src="py", copy_eng=nc.scalar)
                    tr_f += tmpr_pair(0, 2, psum_src="py", copy_eng=nc.scalar)
                    tr_f += tmpr_pair(0, 4, psum_src="py", copy_eng=nc.vector)
                if k > 0:
                    fused_mms(k - 1)
            fused_mms(KT - 1)
            for m in range(FUSED):
                epilogue(0, m, py_f[m], tr_f[m])

            # ---- standard m-pair body: two 32-MM groups with the packed tmpR
            # pair injected mid-group (the deep MM pipeline hides its
            # LDWEIGHTS; at a group boundary it costs a full extra slot) ----
            def body_pair(n, m, wsl, w8n):
                tra = trb = None
                for mm in (m, m + 1):
                    py = py_pool.tile([P, NT], f32, name=f"py{n}_{mm}", tag="py")
                    for k in range(KC):
                        nc.tensor.matmul(
                            py[:],
                            at_view(k)[:, mm * P:(mm + 1) * P],
                            wsl(k),
                            start=(k == 0), stop=False,
                        )
                        if mm == m and k == KT // 2:
                            tra, trb = tmpr_pair(n, m)
                    for jp in range(NPR):
                        nc.tensor.matmul(
                            py[:],
                            at8[jp][:, :, mm * P:(mm + 1) * P],
                            w80n[jp] if w8n is None else w8n[:, jp, :, :],
                            start=False, stop=(jp == NPR - 1),
                            perf_mode=mybir.MatmulPerfMode.DoubleRow,
                        )
                    epilogue(n, mm, py, tra if mm == m else trb)

            w80n = [w80[:, jp, :, :] for jp in range(NPR)]

            # rest of n=0 (w1 prefetch queues behind the prologue loads)
            w1 = w_pool.tile([P, KC, NT], bf16, name="w1", tag="wbig")
            nc.sync.dma_start(w1[:], w_d[1])
            w81 = ev_pool.tile([P, NPR, 2, NT], f8, name="w81", tag="w8", bufs=2)
            nc.sync.dma_start(w81[:], w8_d[1])
            for m in range(FUSED, MT, 2):
                body_pair(0, m, w0slice, None)
            # n = 1..7
            wn, w8n = w1, w81
            for n in range(1, NTS):
                if n + 1 < NTS:
                    wnext = w_pool.tile([P, KC, NT], bf16, name=f"w{n+1}", tag="wbig")
                    nc.sync.dma_start(wnext[:], w_d[n + 1])
                    w8next = ev_pool.tile([P, NPR, 2, NT], f8, name=f"w8{n+1}", tag="w8", bufs=2)
                    nc.sync.dma_start(w8next[:], w8_d[n + 1])
                wcur, w8cur = wn, w8n
                for m in range(0, MT, 2):
                    body_pair(n, m, lambda k, w=wcur: w[:, k, :], w8cur)
                if n + 1 < NTS:
                    wn, w8n = wnext, w8next

    nc.finalize()
    return nc


def _get_program():
    if "nc" not in _CACHE:
        _CACHE["nc"] = _build_program()
    return _CACHE["nc"]


def kernel(x, cluster, weight, style_L, style_R):
    import os

    # The NTFF trace path needs an antenv hook this container lacks; never
    # let a stray BASS_TRACE env take the run down that path.
    os.environ.setdefault("BASS_NEVER_TRACE", "1")
    from concourse.bass_utils import run_bass_kernel_spmd

    nc = _get_program()
    bf16 = ml_dtypes.bfloat16

    # W: [din, dout] -> [n, p, k, nt] partition-major for contiguous DMA.
    # All weights are scaled x64 (exact in bf16; brings the fp8 tail into
    # e4m3 range) and style_R is scaled 1/64 to compensate.
    wsc = np.asarray(weight, dtype=np.float32) * 64.0
    w_full = wsc.reshape(KT, P, NTS, NT)
    w_r = np.ascontiguousarray(
        w_full[:KC].astype(bf16).transpose(2, 1, 0, 3)
    )
    # fp8 tail: [NTS, P, pair, half, NT] e4m3 pairs for DoubleRow
    w_8 = np.ascontiguousarray(
        w_full[KC:].reshape(NPR, 2, P, NTS, NT)
        .transpose(3, 2, 0, 1, 4)
    ).astype(ml_dtypes.float8_e4m3fn)
    # styles/cluster duplicated across both 64-row halves for row packing
    sL1 = np.asarray(style_L, dtype=np.float32).astype(bf16)
    sR1 = (np.asarray(style_R, dtype=np.float32) / 64.0).astype(bf16)
    sL = np.ascontiguousarray(np.vstack([sL1, sL1]))
    sR = np.ascontiguousarray(np.vstack([sR1, sR1]))

    in_maps = []
    for c in range(NCORES):
        xs = np.asarray(x[c * MB:(c + 1) * MB], dtype=np.float32)
        xT = np.ascontiguousarray(xs.T).astype(bf16)          # [DIN, MB]
        # [din, mb] -> [granule, p, k-in-granule, mb]
        xT_r = np.ascontiguousarray(
            xT.reshape(KT // XG, XG, P, MB).transpose(0, 2, 1, 3)
        )
        clT1 = np.ascontiguousarray(
            np.asarray(cluster[c * MB:(c + 1) * MB], dtype=np.float32).T
        ).astype(bf16)
        clT = np.ascontiguousarray(np.vstack([clT1, clT1]))
        in_maps.append(
            {"xT": xT_r, "clusterT": clT, "weight": w_r, "weight8": w_8,
             "style_L": sL, "style_R": sR}
        )

    res = run_bass_kernel_spmd(nc, in_maps, list(range(NCORES)))
    LAST["results"] = res
    LAST["in_maps"] = in_maps
    out = np.concatenate(
        [np.asarray(res.results[c]["out"], dtype=np.float32) for c in range(NCORES)],
        axis=0,
    )
    return out
